# revision 3
# baseline (speedup 1.0000x reference)
"""Trainium2 Bass kernel for nn_GeneralConv2d (binarized 3x3 conv, C=256, s1 p1).

Contract: kernel(**inputs) takes FULL inputs
    x      [32, 256, 56, 56] f32
    weight [589824, 1] f32
returns FULL output y [32, 256, 56, 56] f32.

Strategy: data-parallel over batch across 8 NeuronCores (4 images/core),
weight replicated.  On-device per core:
  scale[o] = mean |w[o]|; sgn = 2*(w>=0)-1 in bf16 (exact +-1)
  36 PE transposes produce lhsT[c,o] bf16 weight tiles
  x cast to bf16 into SBUF with a 1-px zero halo
  conv = 18 accumulating bf16 matmuls per PSUM tile (9 taps x 2 c-chunks),
  evicted with per-out-channel scale on the scalar engine in f32.
"""

from contextlib import ExitStack

import numpy as np

import concourse.bass as bass
import concourse.mybir as mybir
from concourse import bacc
import concourse.tile as tile
from concourse.masks import make_identity

dt = mybir.dt
OUT_C = 256
IN_C = 256
KH = KW = 3
KK = KH * KW           # 9
CKK = IN_C * KK        # 2304
P = 128
CC = IN_C // P         # 2
OO = OUT_C // P        # 2

N_CORES = 8
BATCH = 32
H = W = 56
IMGS = BATCH // N_CORES  # 4 images per core


def _build_conv_nc(imgs: int, h: int, w_: int, hchunk: int, psum_bufs: int = 6):
    assert h % hchunk == 0
    nch = h // hchunk
    hp, wp = h + 2, w_ + 2
    nc = bacc.Bacc("TRN2", target_bir_lowering=False, debug=False,
                   enable_asserts=False, num_devices=8)
    x = nc.declare_dram_parameter("x", [imgs, IN_C, h, w_], dt.float32,
                                  isOutput=False)
    w = nc.declare_dram_parameter("w", [OUT_C * CKK, 1], dt.float32,
                                  isOutput=False)
    y = nc.declare_dram_parameter("y", [imgs, OUT_C, h, w_], dt.float32,
                                  isOutput=True)

    w2d = w.rearrange("(o r) one -> o (r one)", r=CKK)   # [256, 2304]

    with tile.TileContext(nc) as tc, ExitStack() as ctx:
        consts = ctx.enter_context(tc.tile_pool(name="consts", bufs=1))
        ident = consts.tile([P, P], dt.bfloat16)
        make_identity(nc, ident)

        wprep = ctx.enter_context(tc.tile_pool(name="wprep", bufs=1))
        w_sb = wprep.tile([P, OO, CKK], dt.float32)
        # Single DMA so downstream reduce has exactly one producer sem
        # (TensorReduce supports only one hw sync wait).
        w3d = w2d.rearrange("(oo p) r -> p oo r", p=P)
        nc.sync.dma_start(out=w_sb, in_=w3d)

        scale_sb = wprep.tile([P, OO], dt.float32)
        nc.vector.tensor_reduce(
            out=scale_sb, in_=w_sb, axis=mybir.AxisListType.X,
            op=mybir.AluOpType.add, apply_absolute_value=True)
        nc.vector.tensor_scalar_mul(scale_sb, scale_sb, 1.0 / CKK)

        sgn_sb = wprep.tile([P, OO, CKK], dt.bfloat16)
        nc.vector.tensor_scalar(
            out=sgn_sb, in0=w_sb, scalar1=0.0, scalar2=2.0,
            op0=mybir.AluOpType.is_ge, op1=mybir.AluOpType.mult)
        nc.vector.tensor_scalar_add(sgn_sb, sgn_sb, -1.0)
        sgn_v = sgn_sb.rearrange("p oo (c k) -> p oo c k", k=KK)

        tpool = ctx.enter_context(tc.tile_pool(name="tpsum", bufs=2, space="PSUM"))
        wtp = ctx.enter_context(tc.tile_pool(name="wtiles", bufs=OO * CC * KK))
        wt = {}
        for oo in range(OO):
            for cc in range(CC):
                for k in range(KK):
                    tp = tpool.tile([P, P], dt.bfloat16)
                    nc.tensor.transpose(tp, sgn_v[:, oo, cc * P:(cc + 1) * P, k],
                                        ident)
                    t = wtp.tile([P, P], dt.bfloat16)
                    nc.vector.tensor_copy(out=t, in_=tp)
                    wt[(oo, cc, k)] = t

        xp = ctx.enter_context(tc.tile_pool(name="xtiles", bufs=imgs * CC))
        xt = {}
        for img in range(imgs):
            for cc in range(CC):
                t = xp.tile([P, hp, wp], dt.bfloat16)
                nc.vector.memset(t[:, 0, :], 0.0)
                nc.vector.memset(t[:, hp - 1, :], 0.0)
                nc.vector.memset(t[:, :, 0], 0.0)
                nc.vector.memset(t[:, :, wp - 1], 0.0)
                nc.gpsimd.dma_start(out=t[:, 1:h + 1, 1:w_ + 1],
                                    in_=x[img, cc * P:(cc + 1) * P])
                xt[(img, cc)] = t

        pp = ctx.enter_context(
            tc.tile_pool(name="psum", bufs=psum_bufs, space="PSUM"))
        op = ctx.enter_context(tc.tile_pool(name="ostage", bufs=4))
        for img in range(imgs):
            for oo in range(OO):
                for ih in range(nch):
                    ps = pp.tile([P, hchunk * w_], dt.float32)
                    n = 0
                    for cc in range(CC):
                        for k in range(KK):
                            ki, kj = divmod(k, KW)
                            rhs = xt[(img, cc)][
                                :, ih * hchunk + ki: ih * hchunk + ki + hchunk,
                                kj: kj + w_]
                            nc.tensor.matmul(
                                ps, lhsT=wt[(oo, cc, k)], rhs=rhs,
                                start=(n == 0), stop=(n == CC * KK - 1))
                            n += 1
                    st = op.tile([P, hchunk * w_], dt.float32)
                    nc.scalar.mul(st, ps, scale_sb[:, oo:oo + 1])
                    nc.sync.dma_start(
                        out=y[img, oo * P:(oo + 1) * P,
                              ih * hchunk:(ih + 1) * hchunk, :],
                        in_=st)
    nc.compile()
    return nc


_NC_CACHE = {}


def _get_nc():
    key = (IMGS, H, W)
    if key not in _NC_CACHE:
        _NC_CACHE[key] = _build_conv_nc(IMGS, H, W, hchunk=8)
    return _NC_CACHE[key]


def kernel(**inputs) -> np.ndarray:
    from concourse.bass_utils import run_bass_kernel_spmd

    x = np.ascontiguousarray(np.asarray(inputs["x"], dtype=np.float32))
    weight = np.ascontiguousarray(np.asarray(inputs["weight"], dtype=np.float32))
    assert x.shape == (BATCH, IN_C, H, W), x.shape
    assert weight.shape == (OUT_C * CKK, 1), weight.shape

    nc = _get_nc()
    in_maps = [
        {"x": x[c * IMGS:(c + 1) * IMGS], "w": weight}
        for c in range(N_CORES)
    ]
    res = run_bass_kernel_spmd(nc, in_maps, core_ids=list(range(N_CORES)))
    return np.concatenate([res.results[c]["y"] for c in range(N_CORES)], axis=0)


# revision 4
# speedup vs baseline: 1.0252x; 1.0252x over previous
"""Binarized 3x3 conv (GeneralConv2d) on 8 NeuronCores.

y[b,o,h,w] = mean_abs(w[o]) * sum_{c,kh,kw} sign(w[o,c,kh,kw]) * x[b,c,h+kh-1,w+kw-1]

Data-parallel over batch: 4 images per core on 8 cores; the tiny binarized
weight (576KB -> 288KB as bf16 sign) is replicated.  Per core the conv is a
sum of 18 shifted 128x128 GEMMs per output chunk, accumulated in PSUM in
bf16 (exact +-1 weights, bf16-rounded x), scaled by the per-channel mean-abs
on eviction.
"""

import numpy as np

from contextlib import ExitStack

import concourse.bass as bass
import concourse.mybir as mybir
from concourse import bacc
import concourse.tile as tile
from concourse.masks import make_identity

dt = mybir.dt
OUT_C = 256
IN_C = 256
KH = KW = 3
KK = KH * KW           # 9
CKK = IN_C * KK        # 2304
CHK = P128 = 128
P = 128
CC = IN_C // P         # 2 in-channel chunks
OO = OUT_C // P        # 2 out-channel chunks
QC = CKK // CC         # 1152 columns per (oo,cc) quarter


def _build_conv_nc(imgs: int, H: int, W: int, hchunk: int, psum_bufs: int = 6,
                  ostage_bufs: int = 4):
    assert H % hchunk == 0
    nch = H // hchunk
    Hp, Wp = H + 2, W + 2
    nc = bacc.Bacc("TRN2", target_bir_lowering=False, debug=False,
                   enable_asserts=False, num_devices=8)
    x = nc.declare_dram_parameter("x", [imgs, IN_C, H, W], dt.float32, isOutput=False)
    w = nc.declare_dram_parameter("w", [OUT_C * CKK, 1], dt.float32, isOutput=False)
    y = nc.declare_dram_parameter("y", [imgs, OUT_C, H, W], dt.float32, isOutput=True)

    w2d = w.rearrange("(o r) one -> o (r one)", r=CKK)   # [256, 2304]

    with tile.TileContext(nc) as tc, ExitStack() as ctx:
        consts = ctx.enter_context(tc.tile_pool(name="consts", bufs=1))
        ident = consts.tile([P, P], dt.bfloat16)
        make_identity(nc, ident)
        zrow = consts.tile([P, 2 * Wp], dt.bfloat16)
        nc.vector.memset(zrow, 0.0)

        wprep = ctx.enter_context(tc.tile_pool(name="wprep", bufs=1))
        w_sb = wprep.tile([P, OO, CKK], dt.float32)
        sgn_sb = wprep.tile([P, OO, CKK], dt.bfloat16)
        scale_sb = wprep.tile([P, OO], dt.float32)
        sgn_v = sgn_sb.rearrange("p oo (c k) -> p oo c k", k=KK)

        tpool = ctx.enter_context(tc.tile_pool(name="tpsum", bufs=1, space="PSUM"))
        wtp = ctx.enter_context(tc.tile_pool(name="wtiles", bufs=OO * CC * KK))
        xp = ctx.enter_context(tc.tile_pool(name="xtiles", bufs=imgs * CC))

        xt = {}

        def load_x(img):
            for cc in range(CC):
                t = xp.tile([P, Hp, Wp], dt.bfloat16)
                # Halo zeros on the (idle-at-startup) scalar engine, keeping
                # DVE free for the sign ops the transposes wait on.  The
                # interior halo columns (w=57 of row h, w=0 of row h+1) are
                # adjacent in the flat layout, so three contiguous strips
                # cover the whole halo.
                tf = t.rearrange("p h w -> p (h w)")
                nc.scalar.copy(tf[:, 0:Wp], zrow[:, 0:Wp])
                nc.scalar.copy(tf[:, (Hp - 1) * Wp:Hp * Wp], zrow[:, 0:Wp])
                mid = tf[:, Wp - 1:Wp - 1 + (Hp - 1) * Wp].rearrange(
                    "p (h w) -> p h w", w=Wp)[:, :, 0:2]
                nc.scalar.copy(mid, zrow[:, 0:2 * (Hp - 1)].rearrange(
                    "p (h w) -> p h w", w=2))
                # Two half-row DMAs land on different queues -> parallel
                # transfer, halving time-to-ready for the first conv matmul.
                h2 = H // 2
                nc.gpsimd.dma_start(out=t[:, 1:h2 + 1, 1:W + 1],
                                    in_=x[img, cc * P:(cc + 1) * P, 0:h2])
                nc.gpsimd.dma_start(out=t[:, h2 + 1:H + 1, 1:W + 1],
                                    in_=x[img, cc * P:(cc + 1) * P, h2:H])
                xt[(img, cc)] = t

        wt = {}

        def prep_w_quarter(oo, cc):
            # DMA the (oo, cc) quarter of w: rows o=oo*128+p, cols cc*1152..+1152
            nc.sync.dma_start(
                out=w_sb[:, oo, cc * QC:(cc + 1) * QC],
                in_=w2d[oo * P:(oo + 1) * P, cc * QC:(cc + 1) * QC])
            nc.vector.tensor_scalar(
                out=sgn_sb[:, oo, cc * QC:(cc + 1) * QC],
                in0=w_sb[:, oo, cc * QC:(cc + 1) * QC],
                scalar1=0.0, scalar2=2.0,
                op0=mybir.AluOpType.is_ge, op1=mybir.AluOpType.mult)
            nc.vector.tensor_scalar_add(
                sgn_sb[:, oo, cc * QC:(cc + 1) * QC],
                sgn_sb[:, oo, cc * QC:(cc + 1) * QC], -1.0)
            for k in range(KK):
                tp = tpool.tile([P, P], dt.bfloat16)
                nc.tensor.transpose(tp, sgn_v[:, oo, cc * P:(cc + 1) * P, k], ident)
                t = wtp.tile([P, P], dt.bfloat16)
                nc.vector.tensor_copy(out=t, in_=tp)
                wt[(oo, cc, k)] = t

        def reduce_scale(oo):
            # Per-out-channel scale column (runs on DVE behind the conv).
            nc.vector.tensor_reduce(
                out=scale_sb[:, oo:oo + 1], in_=w_sb[:, oo, :],
                axis=mybir.AxisListType.X,
                op=mybir.AluOpType.add, apply_absolute_value=True)
            nc.vector.tensor_scalar_mul(
                scale_sb[:, oo:oo + 1], scale_sb[:, oo:oo + 1], 1.0 / CKK)

        pp = ctx.enter_context(tc.tile_pool(name="psum", bufs=psum_bufs, space="PSUM"))
        op = ctx.enter_context(tc.tile_pool(name="ostage", bufs=ostage_bufs))

        def mm(ps, img, oo, cc, ih, k, n):
            ki, kj = divmod(k, KW)
            rhs = xt[(img, cc)][
                :, ih * hchunk + ki: ih * hchunk + ki + hchunk, kj: kj + W]
            nc.tensor.matmul(ps, lhsT=wt[(oo, cc, k)], rhs=rhs,
                             start=(n == 0), stop=(n == CC * KK - 1))

        def conv_a(img, oo, tiles):
            # Pass A: all cc0 taps for the group's tiles (start accumulation).
            group = {}
            for ih in tiles:
                ps = pp.tile([P, hchunk * W], dt.float32,
                             name=f"ps_{img}_{oo}_{ih}", tag="ps")
                group[ih] = ps
                for k in range(KK):
                    mm(ps, img, oo, 0, ih, k, n=k)
            return group

        def conv_b(img, oo, group):
            # Pass B: cc1 taps, then scale + store.
            for ih, ps in group.items():
                for k in range(KK):
                    mm(ps, img, oo, 1, ih, k, n=KK + k)
                st = op.tile([P, hchunk * W], dt.float32,
                             name=f"st_{img}_{oo}_{ih}", tag="st")
                nc.scalar.mul(st, ps, scale_sb[:, oo:oo + 1])
                nc.sync.dma_start(
                    out=y[img, oo * P:(oo + 1) * P,
                          ih * hchunk:(ih + 1) * hchunk, :],
                    in_=st)

        def conv_rest(img, oo, skip_first=True):
            for g0 in range(psum_bufs if skip_first else 0, nch, psum_bufs):
                tiles = list(range(g0, min(g0 + psum_bufs, nch)))
                conv_b(img, oo, conv_a(img, oo, tiles))

        # Emission order doubles as per-engine program order (PE is in-order):
        # transpose batches alternate with conv half-passes so each batch's
        # DVE-side prep (sign + copies) completes during the previous conv
        # burst and no transpose wait stalls ready conv matmuls behind it.
        load_x(0)
        prep_w_quarter(0, 0)
        g00 = conv_a(0, 0, list(range(min(psum_bufs, nch))))
        prep_w_quarter(0, 1)
        reduce_scale(0)
        if imgs > 1:
            load_x(1)
        conv_b(0, 0, g00)
        prep_w_quarter(1, 0)
        conv_rest(0, 0)
        g01 = conv_a(0, 1, list(range(min(psum_bufs, nch))))
        prep_w_quarter(1, 1)
        reduce_scale(1)
        for img in range(2, imgs):
            load_x(img)
        conv_b(0, 1, g01)
        conv_rest(0, 1)
        for img in range(1, imgs):
            conv_rest(img, 0, skip_first=False)
            conv_rest(img, 1, skip_first=False)
    nc.compile()
    return nc


BATCH, IN_C_, H, W = 32, 256, 56, 56
N_CORES = 8
IMGS = BATCH // N_CORES
_NC_CACHE = {}


def _get_nc():
    key = (IMGS, H, W)
    if key not in _NC_CACHE:
        _NC_CACHE[key] = _build_conv_nc(IMGS, H, W, hchunk=8, psum_bufs=7)
    return _NC_CACHE[key]


def kernel(**inputs) -> np.ndarray:
    from concourse.bass_utils import run_bass_kernel_spmd

    x = np.ascontiguousarray(np.asarray(inputs["x"], dtype=np.float32))
    weight = np.ascontiguousarray(np.asarray(inputs["weight"], dtype=np.float32))
    assert x.shape == (BATCH, IN_C, H, W), x.shape
    assert weight.shape == (OUT_C * CKK, 1), weight.shape

    nc = _get_nc()
    in_maps = [
        {"x": x[c * IMGS:(c + 1) * IMGS], "w": weight}
        for c in range(N_CORES)
    ]
    res = run_bass_kernel_spmd(nc, in_maps, core_ids=list(range(N_CORES)))
    return np.concatenate([res.results[c]["y"] for c in range(N_CORES)], axis=0)


# revision 5
# speedup vs baseline: 1.0479x; 1.0221x over previous
"""Binarized 3x3 conv (GeneralConv2d) on 8 NeuronCores.

y[b,o,h,w] = mean_abs(w[o]) * sum_{c,kh,kw} sign(w[o,c,kh,kw]) * x[b,c,h+kh-1,w+kw-1]

Data-parallel over batch: 4 images per core on 8 cores; the tiny binarized
weight is replicated.  Per core the conv is a sum of 18 shifted 128x128
GEMMs per output chunk, accumulated in PSUM in bf16 (exact +-1 weights,
bf16-rounded x), scaled by the per-channel mean-abs on eviction.
"""

import numpy as np

from contextlib import ExitStack

import concourse.bass as bass
import concourse.mybir as mybir
from concourse import bacc
import concourse.tile as tile
from concourse.masks import make_identity

dt = mybir.dt
OUT_C = 256
IN_C = 256
KH = KW = 3
KK = KH * KW           # 9
CKK = IN_C * KK        # 2304
CHK = P128 = 128
P = 128
CC = IN_C // P         # 2 in-channel chunks
OO = OUT_C // P        # 2 out-channel chunks
QC = CKK // CC         # 1152 columns per (oo,cc) quarter


def _build_conv_nc(imgs: int, H: int, W: int, hchunk: int, psum_bufs: int = 7,
                  ostage_bufs: int = 4, gsz: int = 4, tp_bufs: int = 1):
    assert H % hchunk == 0
    nch = H // hchunk
    Hp, Wp = H + 2, W + 2
    nc = bacc.Bacc("TRN2", target_bir_lowering=False, debug=False,
                   enable_asserts=False, num_devices=8)
    x = nc.declare_dram_parameter("x", [imgs, IN_C, H, W], dt.float32, isOutput=False)
    w = nc.declare_dram_parameter("w", [OUT_C * CKK, 1], dt.float32, isOutput=False)
    y = nc.declare_dram_parameter("y", [imgs, OUT_C, H, W], dt.float32, isOutput=True)

    w2d = w.rearrange("(o r) one -> o (r one)", r=CKK)   # [256, 2304]

    with tile.TileContext(nc) as tc, ExitStack() as ctx:
        consts = ctx.enter_context(tc.tile_pool(name="consts", bufs=1))
        ident = consts.tile([P, P], dt.bfloat16)
        make_identity(nc, ident)
        zrow = consts.tile([P, 2 * Wp], dt.bfloat16)
        nc.vector.memset(zrow, 0.0)

        wprep = ctx.enter_context(tc.tile_pool(name="wprep", bufs=1))
        w_sb = wprep.tile([P, OO, CKK], dt.float32)
        sgn_sb = wprep.tile([P, OO, CKK], dt.bfloat16)
        scale_sb = wprep.tile([P, OO], dt.float32)
        sgn_v = sgn_sb.rearrange("p oo (c k) -> p oo c k", k=KK)

        tpool = ctx.enter_context(tc.tile_pool(name="tpsum", bufs=tp_bufs, space="PSUM"))
        wtp = ctx.enter_context(tc.tile_pool(name="wtiles", bufs=OO * CC * KK))
        xp = ctx.enter_context(tc.tile_pool(name="xtiles", bufs=imgs * CC))

        xt = {}

        def load_x(img):
            for cc in range(CC):
                t = xp.tile([P, Hp, Wp], dt.bfloat16)
                # Halo zeros on the (idle-at-startup) scalar engine, keeping
                # DVE free for the sign ops the transposes wait on.  The
                # interior halo columns (w=57 of row h, w=0 of row h+1) are
                # adjacent in the flat layout, so three contiguous strips
                # cover the whole halo.
                tf = t.rearrange("p h w -> p (h w)")
                nc.scalar.copy(tf[:, 0:Wp], zrow[:, 0:Wp])
                nc.scalar.copy(tf[:, (Hp - 1) * Wp:Hp * Wp], zrow[:, 0:Wp])
                mid = tf[:, Wp - 1:Wp - 1 + (Hp - 1) * Wp].rearrange(
                    "p (h w) -> p h w", w=Wp)[:, :, 0:2]
                nc.scalar.copy(mid, zrow[:, 0:2 * (Hp - 1)].rearrange(
                    "p (h w) -> p h w", w=2))
                # Two half-row DMAs land on different queues -> parallel
                # transfer, halving time-to-ready for the first conv matmul.
                h2 = H // 2
                nc.gpsimd.dma_start(out=t[:, 1:h2 + 1, 1:W + 1],
                                    in_=x[img, cc * P:(cc + 1) * P, 0:h2])
                nc.gpsimd.dma_start(out=t[:, h2 + 1:H + 1, 1:W + 1],
                                    in_=x[img, cc * P:(cc + 1) * P, h2:H])
                xt[(img, cc)] = t

        wt = {}

        def prep_w_quarter(oo, cc):
            # DMA the (oo, cc) quarter of w: rows o=oo*128+p, cols cc*1152..+1152
            q2 = QC // 2
            for h in range(2):
                nc.sync.dma_start(
                    out=w_sb[:, oo, cc * QC + h * q2:cc * QC + (h + 1) * q2],
                    in_=w2d[oo * P:(oo + 1) * P,
                            cc * QC + h * q2:cc * QC + (h + 1) * q2])
            nc.vector.tensor_scalar(
                out=sgn_sb[:, oo, cc * QC:(cc + 1) * QC],
                in0=w_sb[:, oo, cc * QC:(cc + 1) * QC],
                scalar1=0.0, scalar2=2.0,
                op0=mybir.AluOpType.is_ge, op1=mybir.AluOpType.mult)
            nc.vector.tensor_scalar_add(
                sgn_sb[:, oo, cc * QC:(cc + 1) * QC],
                sgn_sb[:, oo, cc * QC:(cc + 1) * QC], -1.0)
            for k in range(KK):
                tp = tpool.tile([P, P], dt.bfloat16)
                nc.tensor.transpose(tp, sgn_v[:, oo, cc * P:(cc + 1) * P, k], ident)
                t = wtp.tile([P, P], dt.bfloat16)
                nc.vector.tensor_copy(out=t, in_=tp)
                wt[(oo, cc, k)] = t

        def reduce_scale(oo):
            # Per-out-channel scale column (runs on DVE behind the conv).
            nc.vector.tensor_reduce(
                out=scale_sb[:, oo:oo + 1], in_=w_sb[:, oo, :],
                axis=mybir.AxisListType.X,
                op=mybir.AluOpType.add, apply_absolute_value=True)
            nc.vector.tensor_scalar_mul(
                scale_sb[:, oo:oo + 1], scale_sb[:, oo:oo + 1], 1.0 / CKK)

        pp = ctx.enter_context(tc.tile_pool(name="psum", bufs=psum_bufs, space="PSUM"))
        op = ctx.enter_context(tc.tile_pool(name="ostage", bufs=ostage_bufs))

        def mm(ps, img, oo, cc, ih, k, n):
            ki, kj = divmod(k, KW)
            rhs = xt[(img, cc)][
                :, ih * hchunk + ki: ih * hchunk + ki + hchunk, kj: kj + W]
            nc.tensor.matmul(ps, lhsT=wt[(oo, cc, k)], rhs=rhs,
                             start=(n == 0), stop=(n == CC * KK - 1))

        def conv_a(img, oo, tiles):
            # Pass A: all cc0 taps for the group's tiles (start accumulation).
            group = {}
            for ih in tiles:
                ps = pp.tile([P, hchunk * W], dt.float32,
                             name=f"ps_{img}_{oo}_{ih}", tag="ps")
                group[ih] = ps
                for k in range(KK):
                    mm(ps, img, oo, 0, ih, k, n=k)
            return group

        def conv_b(img, oo, group):
            # Pass B: cc1 taps, then scale + store.
            for ih, ps in group.items():
                for k in range(KK):
                    mm(ps, img, oo, 1, ih, k, n=KK + k)
                st = op.tile([P, hchunk * W], dt.float32,
                             name=f"st_{img}_{oo}_{ih}", tag="st")
                nc.scalar.mul(st, ps, scale_sb[:, oo:oo + 1])
                nc.sync.dma_start(
                    out=y[img, oo * P:(oo + 1) * P,
                          ih * hchunk:(ih + 1) * hchunk, :],
                    in_=st)

        def conv(img, oo, skip=0):
            for g0 in range(skip, nch, gsz):
                tiles = list(range(g0, min(g0 + gsz, nch)))
                conv_b(img, oo, conv_a(img, oo, tiles))

        # Emission order doubles as per-engine program order (PE is in-order):
        # transpose batches alternate with conv half-passes so each batch's
        # DVE-side prep (sign + copies) completes during the previous conv
        # burst and no transpose wait stalls ready conv matmuls behind it.
        # Groups of `gsz` < psum_bufs keep consecutive groups on disjoint
        # PSUM banks, so pass A never WAR-waits on the previous group's
        # evictions.
        load_x(0)
        prep_w_quarter(0, 0)
        a1 = conv_a(0, 0, list(range(min(gsz, nch))))
        prep_w_quarter(0, 1)
        reduce_scale(0)
        if imgs > 1:
            load_x(1)
        conv_b(0, 0, a1)
        prep_w_quarter(1, 0)
        if nch > gsz:
            a2 = conv_a(0, 0, list(range(gsz, min(2 * gsz, nch))))
            prep_w_quarter(1, 1)
            reduce_scale(1)
            conv_b(0, 0, a2)
            conv(0, 0, skip=2 * gsz)
        else:
            prep_w_quarter(1, 1)
            reduce_scale(1)
        for img in range(2, imgs):
            load_x(img)
        conv(0, 1)
        for img in range(1, imgs):
            conv(img, 0)
            conv(img, 1)
    nc.compile()
    return nc


BATCH, H, W = 32, 56, 56
N_CORES = 8
IMGS = BATCH // N_CORES
_NC_CACHE = {}


def _get_nc():
    key = (IMGS, H, W)
    if key not in _NC_CACHE:
        _NC_CACHE[key] = _build_conv_nc(IMGS, H, W, hchunk=8, psum_bufs=7,
                                        gsz=4, tp_bufs=1)
    return _NC_CACHE[key]


def kernel(**inputs) -> np.ndarray:
    from concourse.bass_utils import run_bass_kernel_spmd

    x = np.ascontiguousarray(np.asarray(inputs["x"], dtype=np.float32))
    weight = np.ascontiguousarray(np.asarray(inputs["weight"], dtype=np.float32))
    assert x.shape == (BATCH, IN_C, H, W), x.shape
    assert weight.shape == (OUT_C * CKK, 1), weight.shape

    nc = _get_nc()
    in_maps = [
        {"x": x[c * IMGS:(c + 1) * IMGS], "w": weight}
        for c in range(N_CORES)
    ]
    res = run_bass_kernel_spmd(nc, in_maps, core_ids=list(range(N_CORES)))
    return np.concatenate([res.results[c]["y"] for c in range(N_CORES)], axis=0)
